# revision 62
# baseline (speedup 1.0000x reference)
"""Trainium2 Bass kernel for nn_MultiHeadAttention (B=4,T=1024,C=1024,H=16).

Sharding: 8 cores = 4 batches x 2 query-halves. Each core computes, for its
batch b and its 512 query rows:
  V projection (natural layout, mask folded in, +mask column for denominator),
  then per head-pair: Q^T/K^T projection chunks, S^T = K^T.T @ Q^T (row-tiled
  head pairs, D=64 contraction), one exp ACT per key chunk over the merged
  two-head S psum, O^T+denominator via one augmented matmul lhsT=[V_h*m | m],
  normalize via reciprocal + DRAM-bounce partition-broadcast; finally
  Y = O^T.T @ Wo with LN stats read straight from PSUM and the LN apply on
  ACT. Keys are mask-sorted host-side so masked tail chunks drop (NK=5 of 8).

Perf notes (HW ~147-152us, median ~150us, from 210us baseline):
  - the PE executes matmuls in program order and its clock drops to a slow
    p-state (~2x) after any stall, re-ramping over ~3us — so emission order
    is scheduling: pair c+1's Q/K matmuls are interleaved into pair c's
    S/exp phase as fill work, and VA/OT/et are split into per-chunk tiles
    so Tile's dependency tracking lets the Wo projection start while late
    pairs still normalize.
  - only sync+scalar have fast (~183 GB/s) HW DMA queues; gpsimd's software
    queue is ~40 GB/s. Inputs are split into chunk-halves across the two
    fast queues, ordered by first use (xT, Wv nn-major, xTq, Wq, Wk, Wo).
    The per-pair reciprocal-broadcast / shift / odd-head DMAs ride the sync
    HW queue — on the gpsimd software queue the 256KB broadcast alone took
    6.4us inside every pair's critical chain.
  - po0/po1 double-buffered so the next pair's O matmuls overlap the
    denominator chain; Wo-projection psums reuse the same tags. Output is
    written bf16 and upcast on the host.
  - gamma/beta/boe work is skipped when the actual input values make them
    no-ops (ln_g==1, ln_b==0, bv@Wo+bo==0), with a general fallback build.
Host gathers the 8 [512,1024] outputs into [4,1024,1024].
"""

import os
import sys

import numpy as np

for _p in ("/opt/trn_rl_repo", "/root/.axon_site/_ro/trn_rl_repo"):
    if os.path.isdir(_p) and _p not in sys.path:
        sys.path.append(_p)

import ml_dtypes  # noqa: E402
import concourse.bass as bass  # noqa: E402
import concourse.mybir as mybir  # noqa: E402
import concourse.tile as tile  # noqa: E402
from concourse import bacc  # noqa: E402
from concourse.bass_utils import run_bass_kernel_spmd  # noqa: E402

BF16 = mybir.dt.bfloat16
F32 = mybir.dt.float32
NPBF16 = ml_dtypes.bfloat16

B, T, C, H = 4, 1024, 1024, 16
D = C // H          # 64
P = 128             # partitions
NC = C // P         # 8 chunks of C
NT = T // P         # 8 chunks of T
TQ = T // 2         # 512 query rows per core
NQ = TQ // P        # 4 query chunks
NPAIR = H // 2      # 8 head pairs
EPS = 1e-5

_CACHE = {}
LAST_RESULTS = None


def _ensure_ntff_hook():
    """Register the axon NTFF profiling hook if the image's antenv lacks it."""
    try:
        import antenv.axon_hooks  # noqa: F401
        return
    except ImportError:
        pass
    try:
        import types

        import antenv
        from trn_agent_boot.trn_boot import _ntff_profile_via_ctypes

        mod = types.ModuleType("antenv.axon_hooks")
        state = {"hook": None}
        mod.set_axon_ntff_profile_hook = lambda h: state.__setitem__("hook", h)
        mod.get_axon_ntff_profile_hook = lambda: state["hook"]
        sys.modules["antenv.axon_hooks"] = mod
        antenv.axon_hooks = mod
        hook = _ntff_profile_via_ctypes("/opt/axon/libaxon_pjrt.so")
        if hook is not None:
            mod.set_axon_ntff_profile_hook(hook)
    except Exception:
        pass


def _emit(nc, tc, dr, NK, use_lnw, use_boe):
    """Emit the per-core Tile program (projections interleaved with attention)."""
    from contextlib import ExitStack

    AF = mybir.ActivationFunctionType
    OP = mybir.AluOpType

    with ExitStack() as ctx:
        consts = ctx.enter_context(tc.tile_pool(name="consts", bufs=1))

        # ---- persistent SBUF tiles ----
        # VA and OT are split into per-chunk tiles so Tile's dependency
        # tracking stays precise: Wo-projection matmuls on OT chunk c must
        # only wait for pair c (not the last pair), which keeps the PE busy
        # and at full p-state through the tail.
        KL = NK * P
        VA = [
            consts.tile([P, H, D + 1], BF16, tag=f"va{j}", name=f"va{j}")
            for j in range(NK)
        ]
        OTc = [
            consts.tile([P, TQ], BF16, tag=f"ot{c}", name=f"ot{c}")
            for c in range(NPAIR)
        ]
        Wo_sb = consts.tile([P, NC, C], BF16)
        vecs = consts.tile([P, NC, 3], F32)        # bq | bk | maskf
        maskv = consts.tile([P, NK], BF16)
        eps_t = consts.tile([P, 1], F32)
        if use_lnw:
            lng_rep = consts.tile([P, C], F32)
            lnb_rep = consts.tile([P, C], F32)
        if use_boe:
            boe_sb = consts.tile([1, C], BF16)     # bv@Wo+bo row (partition 0)
            ones_sb = consts.tile([1, P], BF16)    # ones row for bias preload
            nc.vector.memset(ones_sb, 1.0)

        nc.vector.memset(eps_t, EPS)
        # prefetch the Sqrt activation table while ACT is idle at startup —
        # otherwise its ~1.3us load lands mid-stream in the last pair's phase
        warm = consts.tile([P, 1], F32)
        nc.scalar.activation(
            out=warm[:], in_=eps_t[:],
            func=mybir.ActivationFunctionType.Sqrt,
            bias=eps_t[:], scale=1.0,
        )

        with (
            tc.tile_pool(name="pa", bufs=1) as pa,
            tc.tile_pool(name="pb", bufs=2) as pb,
            tc.tile_pool(name="pbd", bufs=2, space="DRAM") as pbd,
            tc.tile_pool(name="psP", bufs=2, space="PSUM") as psP,
            tc.tile_pool(name="psS", bufs=2, space="PSUM") as psS,
            tc.tile_pool(name="psO", bufs=2, space="PSUM") as psO,
        ):
            xT = pa.tile([P, NC, KL], BF16)
            xTq = pa.tile([P, NC, TQ], BF16)
            Wq_sb = pa.tile([P, NC, C], BF16)
            Wk_sb = pa.tile([P, NC, C], BF16)
            Wv_sb = pa.tile([P, 2, NC, TQ], BF16)  # nn-major

            # ---- input DMAs. Only sync and scalar have fast HW queues
            # (~180 GB/s each); gpsimd's software queue is ~40 GB/s and only
            # carries the small vectors. Each tensor is split into two
            # chunk-halves, one per queue, ordered by first use: V-projection
            # inputs first (the PE program starts with them), Wo last.
            # Wv is nn-major ([P, 2, NC, TQ]) so its nn=0 columns land first.
            HNC = NC // 2
            halves = [
                ("xT", xT, xT), ("Wv0", Wv_sb[:, 0], None),
                ("Wv1", Wv_sb[:, 1], None), ("xTq", xTq, xTq),
                ("Wq", Wq_sb, Wq_sb), ("Wk", Wk_sb, Wk_sb),
                ("Wo", Wo_sb, Wo_sb),
            ]
            for name, tl, _ in halves:
                dname = name[:2] if name.startswith("Wv") else name
                a = dr[dname].ap()
                if name.startswith("Wv"):
                    a = a[:, int(name[2])]
                nc.sync.dma_start(out=tl[:, 0:HNC], in_=a[:, 0:HNC])
                nc.scalar.dma_start(out=tl[:, HNC:], in_=a[:, HNC:])
            nc.gpsimd.dma_start(out=vecs[:], in_=dr["vecs"].ap()[:])
            nc.gpsimd.dma_start(out=maskv[:], in_=dr["maskv"].ap()[:])
            if use_boe:
                nc.gpsimd.dma_start(out=boe_sb[:], in_=dr["boe"].ap()[:])
            if use_lnw:
                for name, rep in (("lng", lng_rep), ("lnb", lnb_rep)):
                    a = dr[name].ap()
                    bcast = bass.AP(
                        tensor=a.tensor, offset=a.offset, ap=[[0, P], [1, C]]
                    )
                    nc.gpsimd.dma_start(out=rep[:], in_=bcast)

            # ---- V projection: natural [KL, C], masked rows, + mask col ----
            # nn-outer so the first 5 groups only need Wv's nn=0 columns.
            for nn in range(2):
                for tcn in range(NK):
                    ps = psP.tile([P, TQ], F32, tag="psp")
                    for kc in range(NC):
                        nc.tensor.matmul(
                            ps[:],
                            xT[:, kc, tcn * P : (tcn + 1) * P],
                            Wv_sb[:, nn, kc, :],
                            start=(kc == 0),
                            stop=(kc == NC - 1),
                        )
                    nc.vector.tensor_scalar_mul(
                        VA[tcn][:, nn * 8 : (nn + 1) * 8, 0:D],
                        ps[:].rearrange("p (h d) -> p h d", h=8),
                        vecs[:, tcn, 2:3],
                    )
            for tcn in range(NK):
                nc.vector.tensor_copy(
                    out=VA[tcn][:, :, D : D + 1],
                    in_=maskv[:, tcn, None].to_broadcast((P, H, 1)),
                )

            # ---- per head-pair: QT/KT projection, S^T, exp, O^T, normalize.
            # The PE executes matmuls strictly in program order, so pair c+1's
            # Q/K projection matmuls are interleaved into pair c's S/exp phase
            # as fill work: while exp(jc) drains the single-buffered S psum,
            # the PE streams Q/K matmuls instead of idling (which would also
            # drop its p-state).
            def emit_qk(c):
                """Allocate pair c's QT/KT tiles; return (QTc, KTc, steps)."""
                QTc = pb.tile([P, TQ], BF16, tag="qtc", name=f"qt{c}")
                KTc = pb.tile([P, KL], BF16, tag="ktc", name=f"kt{c}")
                steps = []
                psq = psP.tile([P, TQ], F32, tag="psp", name=f"psq{c}")
                for kc in range(NC):
                    steps.append(
                        lambda kc=kc: nc.tensor.matmul(
                            psq[:],
                            Wq_sb[:, kc, c * P : (c + 1) * P],
                            xTq[:, kc, :],
                            start=(kc == 0),
                            stop=(kc == NC - 1),
                        )
                    )
                steps.append(
                    lambda: nc.vector.tensor_scalar_add(
                        QTc[:], psq[:], vecs[:, c, 0:1]
                    )
                )
                for ko in range(0, KL, TQ):
                    w = min(TQ, KL - ko)
                    psk = psP.tile([P, TQ], F32, tag="psp", name=f"psk{c}{ko}")
                    for kc in range(NC):
                        steps.append(
                            lambda kc=kc, ko=ko, w=w, psk=psk: nc.tensor.matmul(
                                psk[:, :w],
                                Wk_sb[:, kc, c * P : (c + 1) * P],
                                xT[:, kc, ko : ko + w],
                                start=(kc == 0),
                                stop=(kc == NC - 1),
                            )
                        )
                    steps.append(
                        lambda ko=ko, w=w, psk=psk: nc.vector.tensor_scalar_add(
                            KTc[:, ko : ko + w], psk[:, :w], vecs[:, c, 1:2]
                        )
                    )
                return QTc, KTc, steps

            QTc, KTc, steps = emit_qk(0)
            for st in steps:
                st()
            for c in range(NPAIR):
                h0, h1 = 2 * c, 2 * c + 1
                if c + 1 < NPAIR:
                    nQT, nKT, nsteps = emit_qk(c + 1)
                else:
                    nQT, nKT, nsteps = None, None, []

                # S^T for both heads of the pair into one 2-bank psum tile,
                # exp'd by a single ACT instruction per key chunk. Per-chunk
                # et tiles keep the O-matmul dependencies precise.
                ets = [
                    pb.tile([P, 2, TQ], BF16, tag=f"et{jc}", bufs=2, name=f"et{jc}")
                    for jc in range(NK)
                ]
                # skew fill away from slot 0 (the first exp gates nothing
                # yet) toward the slots that actually stall on exp drains
                nfill = len(nsteps)
                cuts = [0]
                for jc in range(NK):
                    frac = 0 if jc == 0 else jc / (NK - 1)
                    cuts.append(round(nfill * frac))
                for jc in range(NK):
                    js = slice(jc * P, (jc + 1) * P)
                    s01 = psS.tile([P, 2, TQ], F32, tag="s01", bufs=1)
                    nc.tensor.matmul(
                        s01[:, 0, :],
                        KTc[0:D, js],
                        QTc[0:D, :],
                        start=True, stop=True,
                        tile_position=(0, 0),
                    )
                    nc.tensor.matmul(
                        s01[:, 1, :],
                        KTc[D:P, js],
                        QTc[D:P, :],
                        start=True, stop=True,
                        tile_position=(D, 0),
                    )
                    nc.scalar.activation(
                        out=ets[jc][:], in_=s01[:],
                        func=AF.Exp, scale=0.125,
                    )
                    for st in nsteps[cuts[jc] : cuts[jc + 1]]:
                        st()
                for st in nsteps[cuts[NK] :]:
                    st()

                # O^T + denominator: lhsT = [V_h*m | m]  -> psum [65, TQ]
                po0 = psO.tile([P, TQ], F32, tag="po0", bufs=2)
                po1 = psO.tile([P, TQ], F32, tag="po1", bufs=2)
                for jc in range(NK):
                    nc.tensor.matmul(
                        po0[0 : D + 1, :], VA[jc][:, h0, :], ets[jc][:, 0, :],
                        start=(jc == 0), stop=(jc == NK - 1),
                    )
                for jc in range(NK):
                    nc.tensor.matmul(
                        po1[0 : D + 1, :], VA[jc][:, h1, :], ets[jc][:, 1, :],
                        start=(jc == 0), stop=(jc == NK - 1),
                    )

                # d rows live on psum partition 64: copy out, DMA-shift to
                # partition 0 (approx-recip ucode is broken at base!=0),
                # reciprocal, then DRAM-bounce partition broadcast.
                dsb = pb.tile([P, 2 * TQ], F32, tag="dsb")
                dp0 = pb.tile([1, 2 * TQ], F32, tag="dp0")
                rp0 = pb.tile([1, 2 * TQ], F32, tag="rp0")
                rrep = pb.tile([D, 2 * TQ], F32, tag="rrep")
                nc.vector.tensor_copy(
                    out=dsb[D : D + 1, 0:TQ], in_=po0[D : D + 1, :]
                )
                nc.vector.tensor_copy(
                    out=dsb[D : D + 1, TQ:], in_=po1[D : D + 1, :]
                )
                nc.sync.dma_start(out=dp0[0:1, :], in_=dsb[D : D + 1, :])
                nc.vector.reciprocal_approx_fast(out=rp0[:], in_=dp0[:])
                rdram = pbd.tile([1, 2 * TQ], F32, tag="rdram")
                nc.sync.dma_start(out=rdram[:], in_=rp0[0:1, :])
                src = rdram[0:1, :]
                bcast = bass.AP(
                    tensor=src.tensor, offset=src.offset, ap=[[0, D]] + src.ap[1:]
                )
                nc.sync.dma_start(out=rrep[:], in_=bcast)
                # normalize: even head straight into OT, odd staged + DMA shift
                odd = pb.tile([D, TQ], BF16, tag="odd")
                nc.vector.tensor_tensor(
                    OTc[c][0:D, :], po0[0:D, :], rrep[:, 0:TQ], OP.mult
                )
                nc.vector.tensor_tensor(
                    odd[:], po1[0:D, :], rrep[:, TQ:], OP.mult
                )
                nc.sync.dma_start(out=OTc[c][D:P, :], in_=odd[:])
                QTc, KTc = nQT, nKT

            # ---- output projection + LN (boe preloaded via ones-row matmul,
            # stats straight from PSUM, gamma/beta on the Pool engine) ----
            for qc in range(NQ):
                qs = slice(qc * P, (qc + 1) * P)
                pw0 = psO.tile([P, TQ], F32, tag="po0", bufs=2)
                pw1 = psO.tile([P, TQ], F32, tag="po1", bufs=2)
                pse = [pw0[:], pw1[:]]
                if use_boe:
                    for nn in range(2):
                        nc.tensor.matmul(
                            pse[nn][:],
                            ones_sb[:, 0:P],
                            boe_sb[:, nn * TQ : (nn + 1) * TQ],
                            start=True, stop=False,
                        )
                # nn-outer so stats for the first half overlap the second
                # half's matmuls
                stats = pb.tile([P, 2, nc.vector.BN_STATS_DIM], F32, tag="stats")
                mv = pb.tile([P, nc.vector.BN_AGGR_DIM], F32, tag="mv")
                for nn in range(2):
                    for mc in range(NC):
                        nc.tensor.matmul(
                            pse[nn][:],
                            OTc[mc][:, qs],
                            Wo_sb[:, mc, nn * TQ : (nn + 1) * TQ],
                            start=(mc == 0 and not use_boe),
                            stop=(mc == NC - 1),
                        )
                    nc.vector.bn_stats(out=stats[:, nn, :], in_=pse[nn][:])
                nc.vector.bn_aggr(out=mv[:], in_=stats[:])
                rstd = pb.tile([P, 1], F32, tag="rstd")
                nmr = pb.tile([P, 1], F32, tag="nmr")
                nc.scalar.activation(
                    out=rstd[:], in_=mv[:, 1:2],
                    func=AF.Sqrt,
                    bias=eps_t[:], scale=1.0,
                )
                nc.vector.reciprocal(out=rstd[:], in_=rstd[:])
                nc.vector.tensor_scalar(
                    nmr[:], mv[:, 0:1], rstd[:], -1.0,
                    OP.mult, OP.mult,
                )
                Y = pb.tile([P, C], BF16, tag="ysb", bufs=2)
                for nn in range(2):
                    cs = slice(nn * TQ, (nn + 1) * TQ)
                    if nn == 0:
                        nc.scalar.activation(
                            out=Y[:, cs], in_=pse[nn][:],
                            func=AF.Identity,
                            bias=nmr[:], scale=rstd[:],
                        )
                    else:
                        nc.vector.scalar_tensor_tensor(
                            Y[:, cs], pse[nn][:], rstd[:],
                            nmr[:, 0:1].to_broadcast((P, TQ)),
                            OP.mult, OP.add,
                        )
                    if use_lnw:
                        nc.vector.tensor_tensor(
                            Y[:, cs], Y[:, cs], lng_rep[:, cs], OP.mult
                        )
                        nc.gpsimd.tensor_tensor(
                            Y[:, cs], Y[:, cs], lnb_rep[:, cs], OP.add
                        )
                    # per-half output DMA right after its LN apply, spread
                    # across the two fast queues
                    eng = nc.sync if (2 * qc + nn) % 2 == 0 else nc.scalar
                    eng.dma_start(out=dr["y"].ap()[qs, cs], in_=Y[:, cs])


def _build(NK=NT, use_lnw=True, use_boe=True):
    nc = bacc.Bacc("TRN2", target_bir_lowering=False, debug=False, num_devices=8)
    dr = {}
    dr["xT"] = nc.dram_tensor("xT", [P, NC, NK * P], BF16, kind="ExternalInput")
    dr["xTq"] = nc.dram_tensor("xTq", [P, NC, TQ], BF16, kind="ExternalInput")
    for w in ("Wq", "Wk", "Wo"):
        dr[w] = nc.dram_tensor(w, [P, NC, C], BF16, kind="ExternalInput")
    dr["Wv"] = nc.dram_tensor("Wv", [P, 2, NC, TQ], BF16, kind="ExternalInput")
    dr["vecs"] = nc.dram_tensor("vecs", [P, NC, 3], F32, kind="ExternalInput")
    dr["maskv"] = nc.dram_tensor("maskv", [P, NK], BF16, kind="ExternalInput")
    if use_boe:
        dr["boe"] = nc.dram_tensor("boe", [1, C], BF16, kind="ExternalInput")
    if use_lnw:
        for v in ("lng", "lnb"):
            dr[v] = nc.dram_tensor(v, [1, C], F32, kind="ExternalInput")
    dr["y"] = nc.dram_tensor("y", [TQ, C], BF16, kind="ExternalOutput")
    with tile.TileContext(nc) as tc:
        _emit(nc, tc, dr, NK, use_lnw, use_boe)
    nc.compile()
    return nc


def _chunk(a):
    """[C, N] -> [128, C//128, N] with [p, c, n] = a[128c+p, n]."""
    return np.ascontiguousarray(
        a.reshape(NC, P, -1).transpose(1, 0, 2)
    )


def _prep_inputs(inputs):
    f32 = np.float32
    Wq = np.asarray(inputs["Wq"], f32)
    Wk = np.asarray(inputs["Wk"], f32)
    Wv = np.asarray(inputs["Wv"], f32)
    Wo = np.asarray(inputs["Wo"], f32)
    x = np.asarray(inputs["x"], f32)
    mask = np.asarray(inputs["attn_mask"]).reshape(B, T)
    # sort keys so unmasked come first; masked tail chunks are dropped
    perms = [np.argsort(-mask[b], kind="stable") for b in range(B)]
    m1max = max(int(mask[b].sum()) for b in range(B))
    NK = min(NT, max(1, -(-m1max // P)))
    KL = NK * P
    bq = np.asarray(inputs["bq"], f32)
    bk = np.asarray(inputs["bk"], f32)
    bv = np.asarray(inputs["bv"], f32)
    bo = np.asarray(inputs["bo"], f32)
    ln_g = np.asarray(inputs["ln_g"], f32)
    ln_b = np.asarray(inputs["ln_b"], f32)

    boe = (bv @ Wo + bo).astype(f32)
    use_boe = bool(np.any(boe != 0.0))
    use_lnw = bool(np.any(ln_g != 1.0) or np.any(ln_b != 0.0))
    shared = {
        "Wq": _chunk(Wq).astype(NPBF16),
        "Wk": _chunk(Wk).astype(NPBF16),
        "Wv": np.ascontiguousarray(
            _chunk(Wv).reshape(P, NC, 2, TQ).transpose(0, 2, 1, 3)
        ).astype(NPBF16),
        "Wo": _chunk(Wo).astype(NPBF16),
    }
    if use_boe:
        shared["boe"] = boe.reshape(1, C).astype(NPBF16)
    if use_lnw:
        shared["lng"] = ln_g.reshape(1, C).astype(f32)
        shared["lnb"] = ln_b.reshape(1, C).astype(f32)
    in_maps = []
    for core in range(8):
        b, half = core // 2, core % 2
        xt = np.ascontiguousarray(x[b].T)  # [C, T]
        pk = perms[b][:KL]
        mfp = mask[b][pk].astype(f32)     # permuted/truncated key mask
        vcol = np.zeros((P, NC), f32)
        vcol[:, :NK] = mfp.reshape(NK, P).T
        vecs = np.stack([bq.reshape(NC, P).T, bk.reshape(NC, P).T, vcol], axis=-1)
        m = dict(shared)
        m["xT"] = _chunk(np.ascontiguousarray(xt[:, pk])).astype(NPBF16)
        m["xTq"] = _chunk(xt[:, half * TQ : (half + 1) * TQ]).astype(NPBF16)
        m["vecs"] = np.ascontiguousarray(vecs, f32)
        m["maskv"] = np.ascontiguousarray(mfp.reshape(NK, P).T.astype(NPBF16))
        in_maps.append(m)
    return NK, use_lnw, use_boe, in_maps


def kernel(**inputs):
    global LAST_RESULTS
    NK, use_lnw, use_boe, in_maps = _prep_inputs(inputs)
    key = ("nc", NK, use_lnw, use_boe)
    if key not in _CACHE:
        _CACHE[key] = _build(NK=NK, use_lnw=use_lnw, use_boe=use_boe)
    nc = _CACHE[key]

    trace = os.environ.get("KERNEL_TRACE", "0") == "1"
    if trace:
        _ensure_ntff_hook()
    LAST_RESULTS = run_bass_kernel_spmd(
        nc, in_maps, core_ids=list(range(8)), trace=trace
    )
    out = np.empty((B, T, C), np.float32)
    for core in range(8):
        b, half = core // 2, core % 2
        out[b, half * TQ : (half + 1) * TQ, :] = np.asarray(
            LAST_RESULTS.results[core]["y"], dtype=np.float32
        )
    return out
